# revision 21
# baseline (speedup 1.0000x reference)
"""Bior 2x upsampling (zero-interleave + separable 9-tap filter) on 8 TRN2 cores.

Math: y[n] = sum_m h[n+4-2m] x[m] along each spatial axis (SAME zero padding).
Both separable stages are banded matmuls on the TensorEngine:

  stage 1: T1[w, nh] = sum_h X[h, w]  * A[nh, h]   (lhsT = X,  K = h)
  stage 2: Y[nh, nw] = sum_w T1[w, nh] * A[nw, w]  (lhsT = T1, K = w)

with A[n, m] = h[n+4-2m]. Because A is shift invariant, every matmul's rhs is
a column-slice of one of two constant matrices CAx[i,j] = h[j+4-2i],
CBx[i,j] = h[j-256-2i] (both [128, 520]).

Per 128-row output block (1024 cols = 2 PSUM banks), six uniform
[K=128, M=128, N=260] f32r matmuls (see _emit_block):
  bank 0: main tile0 [0,260) start=True | main tile1 [252,512) |
          corner tile2 [252,512) (rhs nonzero only in last 4 cols)
  bank 1: main tile2 [512,772) start=True | corner tile1 [512,772)
          (rhs nonzero only in first 4 cols) | main tile3 [764,1024)

The corner contributions (the 9-tap halo straddling the 128-row K-tile
boundary at each bank edge) are full-shape matmuls with mostly-zero rhs:
uniform MM shape keeps the PE pipeline dense (a tiny N=4 MM measured
~600-800ns vs ~120ns for N=260).

has_written semantics (HW-validated): start=True clears the whole bank's
bits then writes+sets; start=False accumulates where set, overwrites
where not - so overlapping column ranges accumulate with no pre-zeroing.

Sharding: pure data parallel, 2 images per core across 8 cores.
Matmuls run as float32r (tf32-like, ~5e-4 end-to-end rel err, 4x faster
than fp32); set MM_DTYPE = "f32" for ~2e-7 at ~2x total runtime.
"""

import numpy as np

H_TILDE = np.array([0.03782845550699535, -0.02384946501937986, -0.1106244044184226,
                    0.3774028556126536, 0.8526986790094022, 0.3774028556126537,
                    -0.1106244044184226, -0.02384946501937986, 0.03782845550699535],
                   dtype=np.float32)

B_PER_CORE = 2
N_CORES = 8
H = W = 512
HO = WO = 1024

# "f32r" (fast, ~2e-4 rel err) or "f32" (4x slower matmuls, ~1e-6 rel err)
MM_DTYPE = "f32r"
EVAC_MODE = "banksplit"
OUT_RING = "sync"
PSP_BUFS = 3
XP_BUFS = 8
T1P_BUFS = 8
YP_BUFS = 4

_CACHE = {}


def _consts():
    """One [128, 1040] f32 constant: CAx | CBx (each [128, 520]).

    CAx[i, j] = h[j + 4 - 2i], CBx[i, j] = h[j - 256 - 2i]. Slices:
      main  rhs aligned at +0   : cax[:, 0:260]   /  cbx[:, 256:516]
      corner rhs (same N=260)   : cax[:, 256:516] /  cbx[:, 0:260]
    """
    h = H_TILDE
    cax = np.zeros((128, 520), dtype=np.float32)
    cbx = np.zeros((128, 520), dtype=np.float32)
    for i in range(128):
        for j in range(520):
            k = j + 4 - 2 * i
            if 0 <= k <= 8:
                cax[i, j] = h[k]
            k = j - 256 - 2 * i
            if 0 <= k <= 8:
                cbx[i, j] = h[k]
    return np.concatenate([cax, cbx], axis=1)


def _split_multiwaits(nc, mybir):
    """walrus here encodes at most ONE sem-wait per instruction; hoist extras
    onto preceding same-engine nops (sequencer order => identical semantics)."""
    ctr = 0
    for fn in nc.m.functions:
        for bb in fn.blocks:
            out, changed = [], False
            for ins in bb.instructions:
                si = ins.sync_info
                if si is not None and len(si.on_wait) > 1:
                    waits = list(si.on_wait)
                    for w in waits[:-1]:
                        ctr += 1
                        nop = mybir.InstNoOp(name=f"wsplit-{ctr}", ins=[], outs=[])
                        nop.engine = ins.engine
                        nop.sync_info = mybir.SyncInfo(on_wait=[w], on_update=[])
                        out.append(nop)
                    si.on_wait = [waits[-1]]
                    changed = True
                out.append(ins)
            if changed:
                bb.instructions = out
    return ctr


def _emit_block(nc, ps, src, mlo, mhi, cax, cbx, f32r):
    """Emit the 6 uniform [K=128, M=128, N=260] matmuls for one block.

    ps: PSUM [128, 1024]; src: 4 source tiles (partitions = contraction dim);
    mlo:mhi: the 128-wide free-dim slice of the source tiles forming M.
    Corners are full-shape MMs whose rhs is mostly zeros (uniform shape
    keeps the PE pipeline dense; tiny-N MMs measured ~600ns each)."""
    mm = nc.tensor.matmul
    kw = dict(skip_group_check=True)
    mm(ps[:, 0:260], lhsT=src[0][:, mlo:mhi], rhs=cax[:, 0:260],
       start=True, stop=False, **kw)
    mm(ps[:, 252:512], lhsT=src[1][:, mlo:mhi], rhs=cbx[:, 256:516],
       start=False, stop=False, **kw)
    mm(ps[:, 252:512], lhsT=src[2][:, mlo:mhi], rhs=cbx[:, 0:260],
       start=False, stop=False, **kw)
    mm(ps[:, 512:772], lhsT=src[2][:, mlo:mhi], rhs=cax[:, 0:260],
       start=True, stop=False, **kw)
    mm(ps[:, 512:772], lhsT=src[1][:, mlo:mhi], rhs=cax[:, 256:516],
       start=False, stop=False, **kw)
    mm(ps[:, 764:1024], lhsT=src[3][:, mlo:mhi], rhs=cbx[:, 256:516],
       start=False, stop=True, **kw)


def _build_program(reps=1, timing_mode=False, loop_n=None,
                   skip_in=False, skip_out=False, skip_compute=False):
    import concourse.bass as bass
    import concourse.mybir as mybir
    import concourse.tile as tile

    f32 = mybir.dt.float32
    dmm = mybir.dt.float32r if MM_DTYPE == "f32r" else f32

    nc = bass.Bass("TRN2", target_bir_lowering=False, debug=False,
                   num_devices=N_CORES)
    if timing_mode:
        # same dataflow, but keep the big tensors device-internal so the
        # per-call wall isn't dominated by host<->device shipping
        x_d = nc.dram_tensor("x", [B_PER_CORE, H, W], dmm, kind="Internal")
        y_d = nc.dram_tensor("y", [B_PER_CORE, HO, WO], f32, kind="Internal")
        ydummy_d = nc.dram_tensor("ydummy", [1, 4], f32, kind="ExternalOutput")
    else:
        x_d = nc.dram_tensor("x", [B_PER_CORE, H, W], dmm, kind="ExternalInput")
        y_d = nc.dram_tensor("y", [B_PER_CORE, HO, WO], f32, kind="ExternalOutput")
    c_d = nc.dram_tensor("c", [128, 1040], dmm, kind="ExternalInput")

    with tile.TileContext(nc) as tc:
        with tc.tile_pool(name="consts", bufs=1) as constp, \
             tc.tile_pool(name="xp", bufs=XP_BUFS) as xp, \
             tc.tile_pool(name="t1p", bufs=T1P_BUFS) as t1p, \
             tc.tile_pool(name="yp", bufs=YP_BUFS) as yp, \
             tc.tile_pool(name="psp", bufs=PSP_BUFS, space="PSUM") as psp:

            c_t = constp.tile([128, 1040], dmm)
            nc.scalar.dma_start(out=c_t[:], in_=c_d.ap())
            if timing_mode:
                nc.sync.dma_start(out=ydummy_d.ap(),
                                  in_=c_t[0:1, 0:4].bitcast(f32))
            cax = c_t[:, 0:520]
            cbx = c_t[:, 520:1040]

            copy_flip = [0]

            def evac(ps, out_tile):
                mode = EVAC_MODE
                i = copy_flip[0]
                copy_flip[0] += 1
                if mode == "act":
                    nc.scalar.copy(out=out_tile[:], in_=ps[:])
                elif mode == "dve":
                    nc.vector.tensor_copy(out=out_tile[:], in_=ps[:])
                elif mode == "alt":
                    if i % 2 == 0:
                        nc.scalar.copy(out=out_tile[:], in_=ps[:])
                    else:
                        nc.vector.tensor_copy(out=out_tile[:], in_=ps[:])
                elif mode == "rot21":
                    if i % 3 < 2:
                        nc.scalar.copy(out=out_tile[:], in_=ps[:])
                    else:
                        nc.vector.tensor_copy(out=out_tile[:], in_=ps[:])
                elif mode == "banksplit":
                    # ACT bank0, DVE bank1 (different banks, concurrent)
                    nc.scalar.copy(out=out_tile[:, 0:512], in_=ps[:, 0:512])
                    nc.vector.tensor_copy(out=out_tile[:, 512:1024], in_=ps[:, 512:1024])
                else:
                    raise ValueError(mode)

            def body():
                xts = []
                for b in range(B_PER_CORE):
                    xt = []
                    for tp in range(2):
                        x_pair = xp.tile([128, 2, W], dmm, tag="x", name=f"x_{b}_{tp}")
                        if not skip_in:
                            nc.scalar.dma_start(
                                out=x_pair[:],
                                in_=x_d.ap()[b].rearrange(
                                    "(t p) w -> p t w", p=128)[:, 2 * tp:2 * tp + 2, :])
                        else:
                            nc.gpsimd.memset(x_pair[:].bitcast(f32), 0.0)
                        xt.append(x_pair[:, 0, :])
                        xt.append(x_pair[:, 1, :])
                    xts.append(xt)
                for b in range(B_PER_CORE):
                    xt = xts[b]

                    t1 = []
                    for m in range(4):
                        t1m = t1p.tile([128, 1024], dmm, tag="t1", name=f"t1_{b}_{m}")
                        if not skip_compute:
                            ps = psp.tile([128, 1024], f32, tag="ps", name=f"ps1_{b}_{m}")
                            _emit_block(nc, ps, xt, 128 * m, 128 * (m + 1), cax, cbx, dmm)
                            evac(ps, t1m)
                        t1.append(t1m)

                    for rp in range(4):
                        y_pair = yp.tile([128, 2, 1024], f32, tag="y", name=f"y_{b}_{rp}")
                        for j in range(2):
                            r = 2 * rp + j
                            if not skip_compute:
                                ps = psp.tile([128, 1024], f32, tag="ps", name=f"ps2_{b}_{r}")
                                _emit_block(nc, ps, t1, 128 * r, 128 * (r + 1), cax, cbx, dmm)
                                evac(ps, y_pair[:, j, :])
                            else:
                                nc.gpsimd.memset(y_pair[:, j, :], 0.0)
                        if not skip_out:
                            eng = nc.sync if (OUT_RING == "sync" or rp % 2 == 0) else nc.scalar
                            eng.dma_start(
                                out=y_d.ap()[b].rearrange(
                                    "(r p) c -> p r c", p=128)[:, 2 * rp:2 * rp + 2, :],
                                in_=y_pair[:])

            if loop_n is not None:
                with tc.For_i(0, loop_n, 1):
                    body()
            else:
                for _ in range(reps):
                    body()

    _split_multiwaits(nc, mybir)
    return nc


def _get_program():
    if "nc" not in _CACHE:
        _CACHE["nc"] = _build_program()
        _CACHE["c"] = _consts()
    return _CACHE["nc"], _CACHE["c"]


def kernel(image_batch: np.ndarray) -> np.ndarray:
    from concourse.bass_utils import run_bass_kernel_spmd

    nc, c = _get_program()
    x = np.ascontiguousarray(
        np.asarray(image_batch, dtype=np.float32).reshape(16, H, W))
    in_maps = [
        {"x": x[B_PER_CORE * k:B_PER_CORE * (k + 1)], "c": c}
        for k in range(N_CORES)
    ]
    res = run_bass_kernel_spmd(nc, in_maps, core_ids=list(range(N_CORES)))
    out = np.concatenate([r["y"] for r in res.results], axis=0)
    return out.reshape(16, HO, WO, 1)


# revision 25
# speedup vs baseline: 1.0040x; 1.0040x over previous
"""Bior 2x upsampling (zero-interleave + separable 9-tap filter) on 8 TRN2 cores.

Math: y[n] = sum_m h[n+4-2m] x[m] along each spatial axis (SAME zero padding).
Both separable stages are banded matmuls on the TensorEngine:

  stage 1: T1[w, nh] = sum_h X[h, w]  * A[nh, h]   (lhsT = X,  K = h)
  stage 2: Y[nh, nw] = sum_w T1[w, nh] * A[nw, w]  (lhsT = T1, K = w)

with A[n, m] = h[n+4-2m]. Because A is shift invariant, every matmul's rhs is
a column-slice of one of two constant matrices CAx[i,j] = h[j+4-2i],
CBx[i,j] = h[j-256-2i] (both [128, 520]).

Per 128-row output block (1024 cols = 2 PSUM banks), six uniform
[K=128, M=128, N=260] f32r matmuls (see _emit_block):
  bank 0: main tile0 [0,260) start=True | main tile1 [252,512) |
          corner tile2 [252,512) (rhs nonzero only in last 4 cols)
  bank 1: main tile2 [512,772) start=True | corner tile1 [512,772)
          (rhs nonzero only in first 4 cols) | main tile3 [764,1024)

The corner contributions (the 9-tap halo straddling the 128-row K-tile
boundary at each bank edge) are full-shape matmuls with mostly-zero rhs:
uniform MM shape keeps the PE pipeline dense (a tiny N=4 MM measured
~600-800ns vs ~120ns for N=260).

has_written semantics (HW-validated): start=True clears the whole bank's
bits then writes+sets; start=False accumulates where set, overwrites
where not - so overlapping column ranges accumulate with no pre-zeroing.

Sharding: pure data parallel, 2 images per core across 8 cores.
Matmuls run as float32r (tf32-like, ~5e-4 end-to-end rel err, 4x faster
than fp32); set MM_DTYPE = "f32" for ~2e-7 at ~2x total runtime.
"""

import numpy as np

H_TILDE = np.array([0.03782845550699535, -0.02384946501937986, -0.1106244044184226,
                    0.3774028556126536, 0.8526986790094022, 0.3774028556126537,
                    -0.1106244044184226, -0.02384946501937986, 0.03782845550699535],
                   dtype=np.float32)

B_PER_CORE = 2
N_CORES = 8
H = W = 512
HO = WO = 1024

# "f32r" (fast, ~2e-4 rel err) or "f32" (4x slower matmuls, ~1e-6 rel err)
MM_DTYPE = "f32r"
EVAC_MODE = "banksplit"
OUT_RING = "sync"
IN_ENG = "scalar"
STAGGERED = False
MM_ORDER = "banks"
PSP_BUFS = 4
XP_BUFS = 8
T1P_BUFS = 8
YP_BUFS = 4

_CACHE = {}


def _consts():
    """One [128, 1040] f32 constant: CAx | CBx (each [128, 520]).

    CAx[i, j] = h[j + 4 - 2i], CBx[i, j] = h[j - 256 - 2i]. Slices:
      main  rhs aligned at +0   : cax[:, 0:260]   /  cbx[:, 256:516]
      corner rhs (same N=260)   : cax[:, 256:516] /  cbx[:, 0:260]
    """
    h = H_TILDE
    cax = np.zeros((128, 520), dtype=np.float32)
    cbx = np.zeros((128, 520), dtype=np.float32)
    for i in range(128):
        for j in range(520):
            k = j + 4 - 2 * i
            if 0 <= k <= 8:
                cax[i, j] = h[k]
            k = j - 256 - 2 * i
            if 0 <= k <= 8:
                cbx[i, j] = h[k]
    return np.concatenate([cax, cbx], axis=1)


def _split_multiwaits(nc, mybir):
    """walrus here encodes at most ONE sem-wait per instruction; hoist extras
    onto preceding same-engine nops (sequencer order => identical semantics)."""
    ctr = 0
    for fn in nc.m.functions:
        for bb in fn.blocks:
            out, changed = [], False
            for ins in bb.instructions:
                si = ins.sync_info
                if si is not None and len(si.on_wait) > 1:
                    waits = list(si.on_wait)
                    for w in waits[:-1]:
                        ctr += 1
                        nop = mybir.InstNoOp(name=f"wsplit-{ctr}", ins=[], outs=[])
                        nop.engine = ins.engine
                        nop.sync_info = mybir.SyncInfo(on_wait=[w], on_update=[])
                        out.append(nop)
                    si.on_wait = [waits[-1]]
                    changed = True
                out.append(ins)
            if changed:
                bb.instructions = out
    return ctr


def _emit_block(nc, ps, src, mlo, mhi, cax, cbx, f32r, MM_ORDER=None):
    if MM_ORDER is None:
        MM_ORDER = globals()["MM_ORDER"]
    """Emit the 6 uniform [K=128, M=128, N=260] matmuls for one block.

    ps: PSUM [128, 1024]; src: 4 source tiles (partitions = contraction dim);
    mlo:mhi: the 128-wide free-dim slice of the source tiles forming M.
    Corners are full-shape MMs whose rhs is mostly zeros (uniform shape
    keeps the PE pipeline dense; tiny-N MMs measured ~600ns each)."""
    mm = nc.tensor.matmul
    kw = dict(skip_group_check=True)
    if MM_ORDER == "banks":
        mm(ps[:, 0:260], lhsT=src[0][:, mlo:mhi], rhs=cax[:, 0:260],
           start=True, stop=False, **kw)
        mm(ps[:, 252:512], lhsT=src[1][:, mlo:mhi], rhs=cbx[:, 256:516],
           start=False, stop=False, **kw)
        mm(ps[:, 252:512], lhsT=src[2][:, mlo:mhi], rhs=cbx[:, 0:260],
           start=False, stop=False, **kw)
        mm(ps[:, 512:772], lhsT=src[2][:, mlo:mhi], rhs=cax[:, 0:260],
           start=True, stop=False, **kw)
        mm(ps[:, 512:772], lhsT=src[1][:, mlo:mhi], rhs=cax[:, 256:516],
           start=False, stop=False, **kw)
        mm(ps[:, 764:1024], lhsT=src[3][:, mlo:mhi], rhs=cbx[:, 256:516],
           start=False, stop=True, **kw)
    else:  # "paired": same-lhsT MMs adjacent; bank1's first writer is the
           # tile1 corner (start=True overwrites with zeros+corner, then
           # tile2 main accumulates) — identical math via has_written rules
        mm(ps[:, 0:260], lhsT=src[0][:, mlo:mhi], rhs=cax[:, 0:260],
           start=True, stop=False, **kw)
        mm(ps[:, 252:512], lhsT=src[1][:, mlo:mhi], rhs=cbx[:, 256:516],
           start=False, stop=False, **kw)
        mm(ps[:, 512:772], lhsT=src[1][:, mlo:mhi], rhs=cax[:, 256:516],
           start=True, stop=False, **kw)
        mm(ps[:, 252:512], lhsT=src[2][:, mlo:mhi], rhs=cbx[:, 0:260],
           start=False, stop=False, **kw)
        mm(ps[:, 512:772], lhsT=src[2][:, mlo:mhi], rhs=cax[:, 0:260],
           start=False, stop=False, **kw)
        mm(ps[:, 764:1024], lhsT=src[3][:, mlo:mhi], rhs=cbx[:, 256:516],
           start=False, stop=True, **kw)


def _build_program(reps=1, timing_mode=False, loop_n=None,
                   skip_in=False, skip_out=False, skip_compute=False):
    import concourse.bass as bass
    import concourse.mybir as mybir
    import concourse.tile as tile

    f32 = mybir.dt.float32
    dmm = mybir.dt.float32r if MM_DTYPE == "f32r" else f32

    nc = bass.Bass("TRN2", target_bir_lowering=False, debug=False,
                   num_devices=N_CORES)
    if timing_mode:
        # same dataflow, but keep the big tensors device-internal so the
        # per-call wall isn't dominated by host<->device shipping
        x_d = nc.dram_tensor("x", [B_PER_CORE, H, W], dmm, kind="Internal")
        y_d = nc.dram_tensor("y", [B_PER_CORE, HO, WO], f32, kind="Internal")
        ydummy_d = nc.dram_tensor("ydummy", [1, 4], f32, kind="ExternalOutput")
    else:
        x_d = nc.dram_tensor("x", [B_PER_CORE, H, W], dmm, kind="ExternalInput")
        y_d = nc.dram_tensor("y", [B_PER_CORE, HO, WO], f32, kind="ExternalOutput")
    c_d = nc.dram_tensor("c", [128, 1040], dmm, kind="ExternalInput")

    with tile.TileContext(nc) as tc:
        with tc.tile_pool(name="consts", bufs=1) as constp, \
             tc.tile_pool(name="xp", bufs=XP_BUFS) as xp, \
             tc.tile_pool(name="t1p", bufs=T1P_BUFS) as t1p, \
             tc.tile_pool(name="yp", bufs=YP_BUFS) as yp, \
             tc.tile_pool(name="psp", bufs=PSP_BUFS, space="PSUM") as psp:

            c_t = constp.tile([128, 1040], dmm)
            nc.scalar.dma_start(out=c_t[:], in_=c_d.ap())
            if timing_mode:
                nc.sync.dma_start(out=ydummy_d.ap(),
                                  in_=c_t[0:1, 0:4].bitcast(f32))
            cax = c_t[:, 0:520]
            cbx = c_t[:, 520:1040]

            copy_flip = [0]

            def evac(ps, out_tile):
                mode = EVAC_MODE
                i = copy_flip[0]
                copy_flip[0] += 1
                if mode == "act":
                    nc.scalar.copy(out=out_tile[:], in_=ps[:])
                elif mode == "dve":
                    nc.vector.tensor_copy(out=out_tile[:], in_=ps[:])
                elif mode == "alt":
                    if i % 2 == 0:
                        nc.scalar.copy(out=out_tile[:], in_=ps[:])
                    else:
                        nc.vector.tensor_copy(out=out_tile[:], in_=ps[:])
                elif mode == "rot21":
                    if i % 3 < 2:
                        nc.scalar.copy(out=out_tile[:], in_=ps[:])
                    else:
                        nc.vector.tensor_copy(out=out_tile[:], in_=ps[:])
                elif mode == "banksplit":
                    # ACT bank0, DVE bank1 (different banks, concurrent)
                    nc.scalar.copy(out=out_tile[:, 0:512], in_=ps[:, 0:512])
                    nc.vector.tensor_copy(out=out_tile[:, 512:1024], in_=ps[:, 512:1024])
                else:
                    raise ValueError(mode)

            def body():
                xts = []
                for b in range(B_PER_CORE):
                    xt = []
                    for tp in range(2):
                        x_pair = xp.tile([128, 2, W], dmm, tag="x", name=f"x_{b}_{tp}")
                        if not skip_in:
                            ieng = {"scalar": nc.scalar, "sync": nc.sync,
                                    "gpsimd": nc.gpsimd}[IN_ENG]
                            ieng.dma_start(
                                out=x_pair[:],
                                in_=x_d.ap()[b].rearrange(
                                    "(t p) w -> p t w", p=128)[:, 2 * tp:2 * tp + 2, :])
                        else:
                            nc.gpsimd.memset(x_pair[:].bitcast(f32), 0.0)
                        xt.append(x_pair[:, 0, :])
                        xt.append(x_pair[:, 1, :])
                    xts.append(xt)
                for b in range(B_PER_CORE):
                    xt = xts[b]

                    t1 = []
                    for m in range(4):
                        t1m = t1p.tile([128, 1024], dmm, tag="t1", name=f"t1_{b}_{m}")
                        if not skip_compute:
                            ps = psp.tile([128, 1024], f32, tag="ps", name=f"ps1_{b}_{m}")
                            _emit_block(nc, ps, xt, 128 * m, 128 * (m + 1), cax, cbx, dmm)
                            evac(ps, t1m)
                        t1.append(t1m)

                    for rp in range(4):
                        y_pair = yp.tile([128, 2, 1024], f32, tag="y", name=f"y_{b}_{rp}")
                        for j in range(2):
                            r = 2 * rp + j
                            if not skip_compute:
                                ps = psp.tile([128, 1024], f32, tag="ps", name=f"ps2_{b}_{r}")
                                _emit_block(nc, ps, t1, 128 * r, 128 * (r + 1), cax, cbx, dmm)
                                evac(ps, y_pair[:, j, :])
                            else:
                                nc.gpsimd.memset(y_pair[:, j, :], 0.0)
                        if not skip_out:
                            eng = nc.sync if (OUT_RING == "sync" or rp % 2 == 0) else nc.scalar
                            eng.dma_start(
                                out=y_d.ap()[b].rearrange(
                                    "(r p) c -> p r c", p=128)[:, 2 * rp:2 * rp + 2, :],
                                in_=y_pair[:])

            if loop_n is not None:
                with tc.For_i(0, loop_n, 1, staggered_reset=STAGGERED):
                    body()
            else:
                for _ in range(reps):
                    body()

    _split_multiwaits(nc, mybir)
    return nc


def _get_program():
    if "nc" not in _CACHE:
        _CACHE["nc"] = _build_program()
        _CACHE["c"] = _consts()
    return _CACHE["nc"], _CACHE["c"]


def kernel(image_batch: np.ndarray) -> np.ndarray:
    from concourse.bass_utils import run_bass_kernel_spmd

    nc, c = _get_program()
    x = np.ascontiguousarray(
        np.asarray(image_batch, dtype=np.float32).reshape(16, H, W))
    in_maps = [
        {"x": x[B_PER_CORE * k:B_PER_CORE * (k + 1)], "c": c}
        for k in range(N_CORES)
    ]
    res = run_bass_kernel_spmd(nc, in_maps, core_ids=list(range(N_CORES)))
    out = np.concatenate([r["y"] for r in res.results], axis=0)
    return out.reshape(16, HO, WO, 1)


# revision 27
# speedup vs baseline: 1.0646x; 1.0604x over previous
"""Bior 2x upsampling (zero-interleave + separable 9-tap filter) on 8 TRN2 cores.

Math: y[n] = sum_m h[n+4-2m] x[m] along each spatial axis (SAME zero padding).
Both separable stages are banded matmuls on the TensorEngine:

  stage 1: T1[w, nh] = sum_h X[h, w]  * A[nh, h]   (lhsT = X,  K = h)
  stage 2: Y[nh, nw] = sum_w T1[w, nh] * A[nw, w]  (lhsT = T1, K = w)

with A[n, m] = h[n+4-2m]. Because A is shift invariant, every matmul's rhs is
a column-slice of one of two constant matrices CAx[i,j] = h[j+4-2i],
CBx[i,j] = h[j-256-2i] (both [128, 520]).

Per 128-row output block (1024 cols = 2 PSUM banks), six uniform
[K=128, M=128, N=260] f32r matmuls (see _emit_block):
  bank 0: main tile0 [0,260) start=True | main tile1 [252,512) |
          corner tile2 [252,512) (rhs nonzero only in last 4 cols)
  bank 1: main tile2 [512,772) start=True | corner tile1 [512,772)
          (rhs nonzero only in first 4 cols) | main tile3 [764,1024)

The corner contributions (the 9-tap halo straddling the 128-row K-tile
boundary at each bank edge) are full-shape matmuls with mostly-zero rhs:
uniform MM shape keeps the PE pipeline dense (a tiny N=4 MM measured
~600-800ns vs ~120ns for N=260).

has_written semantics (HW-validated): start=True clears the whole bank's
bits then writes+sets; start=False accumulates where set, overwrites
where not - so overlapping column ranges accumulate with no pre-zeroing.

Sharding: pure data parallel, 2 images per core across 8 cores.
Matmuls run as float32r (tf32-like, ~5e-4 end-to-end rel err, 4x faster
than fp32); set MM_DTYPE = "f32" for ~2e-7 at ~2x total runtime.
"""

import numpy as np

H_TILDE = np.array([0.03782845550699535, -0.02384946501937986, -0.1106244044184226,
                    0.3774028556126536, 0.8526986790094022, 0.3774028556126537,
                    -0.1106244044184226, -0.02384946501937986, 0.03782845550699535],
                   dtype=np.float32)

B_PER_CORE = 2
N_CORES = 8
H = W = 512
HO = WO = 1024

# "f32r" (fast, ~2e-4 rel err) or "f32" (4x slower matmuls, ~1e-6 rel err)
MM_DTYPE = "f32r"
EVAC_MODE = "banksplit"
OUT_RING = "sync"
IN_ENG = "scalar"
STAGGERED = False
MM_ORDER = "banks"
PSP_BUFS = 4
XP_BUFS = 8
T1P_BUFS = 12
YP_BUFS = 6
Y_GROUP = 2

_CACHE = {}


def _consts():
    """One [128, 1040] f32 constant: CAx | CBx (each [128, 520]).

    CAx[i, j] = h[j + 4 - 2i], CBx[i, j] = h[j - 256 - 2i]. Slices:
      main  rhs aligned at +0   : cax[:, 0:260]   /  cbx[:, 256:516]
      corner rhs (same N=260)   : cax[:, 256:516] /  cbx[:, 0:260]
    """
    h = H_TILDE
    cax = np.zeros((128, 520), dtype=np.float32)
    cbx = np.zeros((128, 520), dtype=np.float32)
    for i in range(128):
        for j in range(520):
            k = j + 4 - 2 * i
            if 0 <= k <= 8:
                cax[i, j] = h[k]
            k = j - 256 - 2 * i
            if 0 <= k <= 8:
                cbx[i, j] = h[k]
    return np.concatenate([cax, cbx], axis=1)


def _split_multiwaits(nc, mybir):
    """walrus here encodes at most ONE sem-wait per instruction; hoist extras
    onto preceding same-engine nops (sequencer order => identical semantics)."""
    ctr = 0
    for fn in nc.m.functions:
        for bb in fn.blocks:
            out, changed = [], False
            for ins in bb.instructions:
                si = ins.sync_info
                if si is not None and len(si.on_wait) > 1:
                    waits = list(si.on_wait)
                    for w in waits[:-1]:
                        ctr += 1
                        nop = mybir.InstNoOp(name=f"wsplit-{ctr}", ins=[], outs=[])
                        nop.engine = ins.engine
                        nop.sync_info = mybir.SyncInfo(on_wait=[w], on_update=[])
                        out.append(nop)
                    si.on_wait = [waits[-1]]
                    changed = True
                out.append(ins)
            if changed:
                bb.instructions = out
    return ctr


def _emit_block(nc, ps, src, mlo, mhi, cax, cbx, f32r, MM_ORDER=None):
    if MM_ORDER is None:
        MM_ORDER = globals()["MM_ORDER"]
    """Emit the 6 uniform [K=128, M=128, N=260] matmuls for one block.

    ps: PSUM [128, 1024]; src: 4 source tiles (partitions = contraction dim);
    mlo:mhi: the 128-wide free-dim slice of the source tiles forming M.
    Corners are full-shape MMs whose rhs is mostly zeros (uniform shape
    keeps the PE pipeline dense; tiny-N MMs measured ~600ns each)."""
    mm = nc.tensor.matmul
    kw = dict(skip_group_check=True)
    if MM_ORDER == "banks":
        mm(ps[:, 0:260], lhsT=src[0][:, mlo:mhi], rhs=cax[:, 0:260],
           start=True, stop=False, **kw)
        mm(ps[:, 252:512], lhsT=src[1][:, mlo:mhi], rhs=cbx[:, 256:516],
           start=False, stop=False, **kw)
        mm(ps[:, 252:512], lhsT=src[2][:, mlo:mhi], rhs=cbx[:, 0:260],
           start=False, stop=False, **kw)
        mm(ps[:, 512:772], lhsT=src[2][:, mlo:mhi], rhs=cax[:, 0:260],
           start=True, stop=False, **kw)
        mm(ps[:, 512:772], lhsT=src[1][:, mlo:mhi], rhs=cax[:, 256:516],
           start=False, stop=False, **kw)
        mm(ps[:, 764:1024], lhsT=src[3][:, mlo:mhi], rhs=cbx[:, 256:516],
           start=False, stop=True, **kw)
    else:  # "paired": same-lhsT MMs adjacent; bank1's first writer is the
           # tile1 corner (start=True overwrites with zeros+corner, then
           # tile2 main accumulates) — identical math via has_written rules
        mm(ps[:, 0:260], lhsT=src[0][:, mlo:mhi], rhs=cax[:, 0:260],
           start=True, stop=False, **kw)
        mm(ps[:, 252:512], lhsT=src[1][:, mlo:mhi], rhs=cbx[:, 256:516],
           start=False, stop=False, **kw)
        mm(ps[:, 512:772], lhsT=src[1][:, mlo:mhi], rhs=cax[:, 256:516],
           start=True, stop=False, **kw)
        mm(ps[:, 252:512], lhsT=src[2][:, mlo:mhi], rhs=cbx[:, 0:260],
           start=False, stop=False, **kw)
        mm(ps[:, 512:772], lhsT=src[2][:, mlo:mhi], rhs=cax[:, 0:260],
           start=False, stop=False, **kw)
        mm(ps[:, 764:1024], lhsT=src[3][:, mlo:mhi], rhs=cbx[:, 256:516],
           start=False, stop=True, **kw)


def _build_program(reps=1, timing_mode=False, loop_n=None,
                   skip_in=False, skip_out=False, skip_compute=False):
    import concourse.bass as bass
    import concourse.mybir as mybir
    import concourse.tile as tile

    f32 = mybir.dt.float32
    dmm = mybir.dt.float32r if MM_DTYPE == "f32r" else f32

    nc = bass.Bass("TRN2", target_bir_lowering=False, debug=False,
                   num_devices=N_CORES)
    if timing_mode:
        # same dataflow, but keep the big tensors device-internal so the
        # per-call wall isn't dominated by host<->device shipping
        x_d = nc.dram_tensor("x", [B_PER_CORE, H, W], dmm, kind="Internal")
        y_d = nc.dram_tensor("y", [B_PER_CORE, HO, WO], f32, kind="Internal")
        ydummy_d = nc.dram_tensor("ydummy", [1, 4], f32, kind="ExternalOutput")
    else:
        x_d = nc.dram_tensor("x", [B_PER_CORE, H, W], dmm, kind="ExternalInput")
        y_d = nc.dram_tensor("y", [B_PER_CORE, HO, WO], f32, kind="ExternalOutput")
    c_d = nc.dram_tensor("c", [128, 1040], dmm, kind="ExternalInput")

    with tile.TileContext(nc) as tc:
        with tc.tile_pool(name="consts", bufs=1) as constp, \
             tc.tile_pool(name="xp", bufs=XP_BUFS) as xp, \
             tc.tile_pool(name="t1p", bufs=T1P_BUFS) as t1p, \
             tc.tile_pool(name="yp", bufs=YP_BUFS) as yp, \
             tc.tile_pool(name="psp", bufs=PSP_BUFS, space="PSUM") as psp:

            c_t = constp.tile([128, 1040], dmm)
            nc.scalar.dma_start(out=c_t[:], in_=c_d.ap())
            if timing_mode:
                nc.sync.dma_start(out=ydummy_d.ap(),
                                  in_=c_t[0:1, 0:4].bitcast(f32))
            cax = c_t[:, 0:520]
            cbx = c_t[:, 520:1040]

            copy_flip = [0]

            def evac(ps, out_tile):
                mode = EVAC_MODE
                i = copy_flip[0]
                copy_flip[0] += 1
                if mode == "act":
                    nc.scalar.copy(out=out_tile[:], in_=ps[:])
                elif mode == "dve":
                    nc.vector.tensor_copy(out=out_tile[:], in_=ps[:])
                elif mode == "alt":
                    if i % 2 == 0:
                        nc.scalar.copy(out=out_tile[:], in_=ps[:])
                    else:
                        nc.vector.tensor_copy(out=out_tile[:], in_=ps[:])
                elif mode == "rot21":
                    if i % 3 < 2:
                        nc.scalar.copy(out=out_tile[:], in_=ps[:])
                    else:
                        nc.vector.tensor_copy(out=out_tile[:], in_=ps[:])
                elif mode == "banksplit":
                    # ACT bank0, DVE bank1 (different banks, concurrent)
                    nc.scalar.copy(out=out_tile[:, 0:512], in_=ps[:, 0:512])
                    nc.vector.tensor_copy(out=out_tile[:, 512:1024], in_=ps[:, 512:1024])
                else:
                    raise ValueError(mode)

            def body():
                xts = []
                for b in range(B_PER_CORE):
                    xt = []
                    for tp in range(2):
                        x_pair = xp.tile([128, 2, W], dmm, tag="x", name=f"x_{b}_{tp}")
                        if not skip_in:
                            ieng = {"scalar": nc.scalar, "sync": nc.sync,
                                    "gpsimd": nc.gpsimd}[IN_ENG]
                            ieng.dma_start(
                                out=x_pair[:],
                                in_=x_d.ap()[b].rearrange(
                                    "(t p) w -> p t w", p=128)[:, 2 * tp:2 * tp + 2, :])
                        else:
                            nc.gpsimd.memset(x_pair[:].bitcast(f32), 0.0)
                        xt.append(x_pair[:, 0, :])
                        xt.append(x_pair[:, 1, :])
                    xts.append(xt)
                for b in range(B_PER_CORE):
                    xt = xts[b]

                    t1 = []
                    for m in range(4):
                        t1m = t1p.tile([128, 1024], dmm, tag="t1", name=f"t1_{b}_{m}")
                        if not skip_compute:
                            ps = psp.tile([128, 1024], f32, tag="ps", name=f"ps1_{b}_{m}")
                            _emit_block(nc, ps, xt, 128 * m, 128 * (m + 1), cax, cbx, dmm)
                            evac(ps, t1m)
                        t1.append(t1m)

                    for rp in range(8 // Y_GROUP):
                        y_pair = yp.tile([128, Y_GROUP, 1024], f32, tag="y",
                                         name=f"y_{b}_{rp}")
                        for j in range(Y_GROUP):
                            r = Y_GROUP * rp + j
                            if not skip_compute:
                                ps = psp.tile([128, 1024], f32, tag="ps", name=f"ps2_{b}_{r}")
                                _emit_block(nc, ps, t1, 128 * r, 128 * (r + 1), cax, cbx, dmm)
                                evac(ps, y_pair[:, j, :])
                            else:
                                nc.gpsimd.memset(y_pair[:, j, :], 0.0)
                        if not skip_out:
                            eng = nc.sync if (OUT_RING == "sync" or rp % 2 == 0) else nc.scalar
                            eng.dma_start(
                                out=y_d.ap()[b].rearrange(
                                    "(r p) c -> p r c", p=128)[
                                        :, Y_GROUP * rp:Y_GROUP * (rp + 1), :],
                                in_=y_pair[:])

            if loop_n is not None:
                with tc.For_i(0, loop_n, 1, staggered_reset=STAGGERED):
                    body()
            else:
                for _ in range(reps):
                    body()

    _split_multiwaits(nc, mybir)
    return nc


def _get_program():
    if "nc" not in _CACHE:
        _CACHE["nc"] = _build_program()
        _CACHE["c"] = _consts()
    return _CACHE["nc"], _CACHE["c"]


def kernel(image_batch: np.ndarray) -> np.ndarray:
    from concourse.bass_utils import run_bass_kernel_spmd

    nc, c = _get_program()
    x = np.ascontiguousarray(
        np.asarray(image_batch, dtype=np.float32).reshape(16, H, W))
    in_maps = [
        {"x": x[B_PER_CORE * k:B_PER_CORE * (k + 1)], "c": c}
        for k in range(N_CORES)
    ]
    res = run_bass_kernel_spmd(nc, in_maps, core_ids=list(range(N_CORES)))
    out = np.concatenate([r["y"] for r in res.results], axis=0)
    return out.reshape(16, HO, WO, 1)
